# revision 32
# baseline (speedup 1.0000x reference)
"""Trainium2 Bass kernel for nn_Attention_81776177315877.

Separable-conv attention block (CMT/PVT style):
  x (B=8, 3136, 256) -> q/k/v = sepconv(dw3x3+BN+pw1x1, k/v stride 2)
  -> 8-head attention (d=32) -> proj.

Sharding: data-parallel over batch, core b <- batch b. No collectives.

v6 design (per core):
  - depthwise 3x3 on DVE (9 chained per-partition-scalar MACs, bf16).
    All tap windows are 4-byte aligned via host-side shifted/split copies
    of x (x_pad/x_psh for q; x_e/x_o/x_es for stride-2 k/v) so the DVE
    2x 16-bit mode can engage.
  - pointwise is a true K=256 matmul; v's pointwise runs data-as-weights
    so it directly yields token-major v tiles (no PE transposes).
  - v tiles carry a 33rd column of ones: the O-wave (M=33) computes the
    softmax denominator as a by-product, eliminating the D-wave entirely
    (1/3 of the attention matmul columns).
  - o/d normalize: DVE reciprocal on the psum tile, d-rows extracted by
    ACT copies, GpSimd partition_broadcast replicates, DVE multiplies.
  - rank-1 (K=1) PE matmuls preload biases into PSUM accumulation, so
    PSUM->SBUF staging is a plain ACT copy.
  - exp split ACT (native) / DVE (Schraudolph bitcast), S runs one jt
    ahead.
"""

import sys

sys.path.insert(0, "/opt/trn_rl_repo")

import numpy as np
import ml_dtypes

import concourse.bass as bass
import concourse.bacc as bacc
import concourse.mybir as mybir
import concourse.tile as tile
from concourse.bass_utils import run_bass_kernel_spmd

FP = mybir.dt.float32
BF = mybir.dt.bfloat16
I16 = mybir.dt.int16
AF = mybir.ActivationFunctionType
ALU = mybir.AluOpType

C = 256
HEADS = 8
D = 32
HH = 56
N = HH * HH          # 3136 query tokens
HK = 28
NK = HK * HK         # 784 key tokens
PADW = HH + 2        # 58
KW = 30              # padded width of x_e/x_o/x_es and dwk/dwv rows
EPS = 1e-5
SCALE = D ** -0.5

IC_CH = 8            # query rows per chunk -> 448 free
IC_F = IC_CH * HH    # 448
N_IC = HH // IC_CH   # 7
JT = 112             # key tile (partitions) for attention
N_JT = NK // JT      # 7
VROWS = JT // HK     # 4 dw rows of v per jt chunk

# Schraudolph fast exp in bf16 bits: i16 = A_EXP * S + B_EXP, bits -> bf16
A_EXP = 184.6649652 * SCALE
B_EXP = 16250.5

_CACHED = {}


def _exp_eng(hg, jt, p2):
    """Exp tile engine: 'A' (ACT native) or 'D' (DVE Schraudolph)."""
    if p2 == 0 or jt in (0, 1, 2, 3, 4, 5):
        return "A"
    return "D"


def _build_nc():
    nc = bacc.Bacc("TRN2", target_bir_lowering=False, debug=False, num_devices=8)

    xpad_d = nc.dram_tensor("x_pad", [128, 2, PADW, PADW], BF, kind="ExternalInput")
    dww_d = {}
    pww_d = {}
    for p in ("q", "k", "v"):
        dww_d[p] = nc.dram_tensor(f"{p}_dww", [128, 2, 9], FP, kind="ExternalInput")
        pww_d[p] = nc.dram_tensor(f"{p}_pww", [128, 2, C], BF, kind="ExternalInput")
    const_d = {}
    for p in ("q", "k"):
        const_d[p] = nc.dram_tensor(f"{p}_const", [C, 1], FP, kind="ExternalInput")
    const_d["v"] = nc.dram_tensor("v_const_r1", [1, C], BF, kind="ExternalInput")
    pb_d = nc.dram_tensor("proj_brep", [128, C], FP, kind="ExternalInput")
    pwT_d = nc.dram_tensor("proj_wT", [2, 128, C], BF, kind="ExternalInput")
    out_d = nc.dram_tensor("out", [N, C], FP, kind="ExternalOutput")

    with tile.TileContext(nc) as tc:
        with (
            tc.tile_pool(name="persist", bufs=1) as pp,
            tc.tile_pool(name="ep", bufs=8) as ep,
            tc.tile_pool(name="rp", bufs=2) as rp,
            tc.tile_pool(name="op", bufs=3) as otp,
            tc.tile_pool(name="psS", bufs=3, space="PSUM") as psS,
            tc.tile_pool(name="psA", bufs=1, space="PSUM") as psA,
            tc.tile_pool(name="psB", bufs=1, space="PSUM") as psB,
        ):
            # ---- input / weight loads ----
            dww = {}
            pww = {}
            for p in ("k", "v", "q"):
                dww[p] = pp.tile([128, 2, 9], FP, tag=f"dww{p}", name=f"dww{p}")
                pww[p] = pp.tile([128, 2, C], BF, tag=f"pww{p}", name=f"pww{p}")
            nc.sync.dma_start(dww["k"][:], dww_d["k"][:])
            nc.sync.dma_start(dww["v"][:], dww_d["v"][:])
            nc.sync.dma_start(dww["q"][:], dww_d["q"][:])
            x_pad = pp.tile([128, 2, PADW, PADW], BF, tag="xpad", name="xpad")
            nc.scalar.dma_start(x_pad[:, 0], xpad_d[:, 0])
            nc.scalar.dma_start(x_pad[:, 1], xpad_d[:, 1])
            for p in ("k", "v", "q"):
                nc.gpsimd.dma_start(pww[p][:], pww_d[p][:])
            consts = {}
            for p in ("q", "k"):
                consts[p] = [
                    pp.tile([128, 1], FP, tag=f"const_{p}{cb}", name=f"const_{p}{cb}")
                    for cb in range(2)
                ]
                for cb in range(2):
                    nc.gpsimd.dma_start(consts[p][cb][:], const_d[p][cb * 128:(cb + 1) * 128, :])
            consts["v"] = pp.tile([1, C], BF, tag="const_v", name="const_v")
            nc.gpsimd.dma_start(consts["v"][:], const_d["v"][:])
            pb_rep = pp.tile([128, C], FP, tag="pbrep", name="pbrep")
            nc.gpsimd.dma_start(pb_rep[:], pb_d[:])
            pwT = [pp.tile([128, C], BF, tag=f"pwT{cb}", name=f"pwT{cb}") for cb in range(2)]
            for cb in range(2):
                nc.gpsimd.dma_start(pwT[cb][:], pwT_d[cb, :, :])
            ones_r1 = pp.tile([1, 512], BF, tag="ones_r1", name="ones_r1")
            nc.vector.memset(ones_r1[:], 1.0)

            # stride-2 view of x_pad: (row, parity, col, parity)
            xp2 = x_pad[:].rearrange("p b (ho a) (wv c) -> p b ho a wv c", a=2, c=2)

            # ---- depthwise helpers (all DVE) ----
            def dw_kv(p, dst, ro0, ro1):
                """stride-2 depthwise rows [ro0, ro1): out[ro,wo] =
                sum_t w[t] x_pad[1+2ro+dh, 1+2wo+dwd]."""
                nr = ro1 - ro0
                for cbi in range(2):
                    first = True
                    for tap in range(9):
                        dh, dwd = tap // 3 - 1, tap % 3 - 1
                        r0 = 1 + 2 * ro0 + dh
                        c0 = 1 + dwd
                        src = xp2[:, cbi, r0 // 2: r0 // 2 + nr, r0 % 2,
                                  c0 // 2: c0 // 2 + HK, c0 % 2]
                        w = dww[p][:, cbi, tap:tap + 1]
                        dd = dst[:, cbi, ro0:ro1, :]
                        if first:
                            nc.vector.tensor_scalar(dd, src, w, None, ALU.mult)
                            first = False
                        else:
                            nc.vector.scalar_tensor_tensor(dd, src, w, dd, ALU.mult, ALU.add)

            def dw_q(dst, ro0, ro1, cbis=(0, 1)):
                """stride-1 depthwise rows [ro0, ro1)."""
                for cbi in cbis:
                    first = True
                    for tap in range(9):
                        dh, dwd = tap // 3 - 1, tap % 3 - 1
                        src = x_pad[:, cbi, 1 + ro0 + dh:1 + ro1 + dh, 1 + dwd:1 + dwd + HH]
                        w = dww["q"][:, cbi, tap:tap + 1]
                        dd = dst[:, cbi, ro0:ro1, :]
                        if first:
                            nc.vector.tensor_scalar(dd, src, w, None, ALU.mult)
                            first = False
                        else:
                            nc.vector.scalar_tensor_tensor(dd, src, w, dd, ALU.mult, ALU.add)

            dwk = pp.tile([128, 2, HK, HK], BF, tag="dwk", name="dwk")
            dwv = pp.tile([128, 2, HK, HK], BF, tag="dwv", name="dwv")
            dwq = pp.tile([128, 2, HH, HH], BF, tag="dwq", name="dwq")

            k_cm = [pp.tile([128, NK], BF, tag=f"kcm{cb}", name=f"kcm{cb}") for cb in range(2)]
            q_cm = [pp.tile([128, N], BF, tag=f"qcm{cb}", name=f"qcm{cb}") for cb in range(2)]
            o_cm = [pp.tile([128, N], BF, tag=f"ocm{cb}", name=f"ocm{cb}") for cb in range(2)]
            # v token-major with 32 ones columns per head: the O-wave (M=64)
            # emits the softmax denominator pre-replicated in rows 32:64
            v_tm = pp.tile([128, N_JT, 8, 64], BF, tag="vtm", name="vtm")
            nc.vector.memset(v_tm[:, :, :, 32:64], 1.0)

            # ---- pointwise helpers (PE) ----
            def pw_cm(p, dst_tiles, src, t0, t1, src_rows, width):
                """channel-major pointwise with rank-1 bias preload.
                Token range [t0, t1) in units of `width`-wide rows."""
                fsz = t1 - t0
                for half in range(2):
                    cps = (psA if half == 0 else psB).tile(
                        [128, 448], FP, tag=f"ps{'AB'[half]}", name="cps"
                    )
                    for cbi in range(2):
                        nc.tensor.matmul(
                            cps[:, :fsz],
                            lhsT=(pww[p][:, cbi, half * 128:(half + 1) * 128]),
                            rhs=(src[:, cbi, src_rows[0]:src_rows[1], 0:width]),
                            start=(cbi == 0),
                            stop=(cbi == 1),
                        )
                    nc.scalar.activation(
                        dst_tiles[half][:, t0:t1], cps[:, :fsz],
                        AF.Identity, bias=consts[p][half],
                    )

            def pw_v(jt):
                """token-major pointwise for v -> v_tm[0:112, jt, :, 0:32]."""
                vps = (psA if jt % 2 == 0 else psB).tile(
                    [128, 448], FP, tag=f"ps{'AB'[jt % 2]}", name="vps"
                )
                nc.tensor.matmul(
                    vps[:JT, :C],
                    lhsT=(ones_r1[:, :JT]),
                    rhs=(consts["v"][:, :]),
                    start=True,
                    stop=False,
                )
                dv = dwv[:].rearrange("p b r w -> p b (r w)")
                for cbi in range(2):
                    nc.tensor.matmul(
                        vps[:JT, :C],
                        lhsT=(dv[:, cbi, jt * JT:(jt + 1) * JT]),
                        rhs=(pww["v"][:, cbi, :]),
                        start=False,
                        stop=(cbi == 1),
                    )
                nc.scalar.activation(v_tm[:JT, jt, :, 0:32], vps[:JT, :C], AF.Copy)

            # ---- startup (ordered so attention can begin after k-half + q0) ----
            dw_kv("k", dwk, 0, 14)
            pw_cm("k", k_cm, dwk, 0, 14 * HK, (0, 14), HK)
            dw_q(dwq, 0, IC_CH)
            pw_cm("q", q_cm, dwq, 0, IC_F, (0, IC_CH), HH)
            dw_kv("k", dwk, 14, HK)
            pw_cm("k", k_cm, dwk, 14 * HK, NK, (14, HK), HK)
            dw_kv("v", dwv, 0, HK)
            for jt in range(N_JT):
                pw_v(jt)
            dw_q(dwq, IC_CH, 2 * IC_CH)

            # ---- main loop ----
            n_tt = (N + 127) // 128  # 25 output token blocks
            ti_ready = 0

            def proj_block(st):
                pps = psB.tile([128, 448], FP, tag="psB", name="pj")
                for cb in range(2):
                    nc.tensor.matmul(
                        pps[:, :C],
                        lhsT=(o_cm[cb][:, st:st + 128]),
                        rhs=(pwT[cb][:, :]),
                        start=(cb == 0),
                        stop=(cb == 1),
                    )
                ot = otp.tile([128, C], FP, tag="ot", name="ot")
                nc.vector.scalar_tensor_tensor(
                    ot[:], pps[:, :C], 1.0, pb_rep[:], ALU.mult, ALU.add
                )
                nc.sync.dma_start(out_d[st:st + 128, :], ot[:])

            for ic in range(N_IC):
                for hg in range(2):
                    o_psA = psA.tile([128, 448], FP, tag="psA", name="oA")
                    o_psB = psB.tile([128, 448], FP, tag="psB", name="oB")
                    o_ps4 = {0: (o_psA, 0), 1: (o_psA, 64), 2: (o_psB, 0), 3: (o_psB, 64)}

                    def s_pair(jt, p2):
                        s4p = psS.tile([112, 2, 512], FP, tag="s4", name="s4")
                        for hh in (2 * p2, 2 * p2 + 1):
                            nc.tensor.matmul(
                                s4p[:JT, hh % 2, :IC_F],
                                lhsT=(k_cm[hg][hh * 32:(hh + 1) * 32, jt * JT:(jt + 1) * JT]),
                                rhs=(q_cm[hg][hh * 32:(hh + 1) * 32, ic * IC_F:(ic + 1) * IC_F]),
                                start=True,
                                stop=True,
                                tile_position=(32 * hh, 0),
                                skip_group_check=True,
                            )
                        return s4p

                    s4s = [s_pair(0, 0), s_pair(0, 1)]
                    for jt in range(N_JT):
                        e4p = [None, None]
                        for p2 in range(2):
                            e4 = ep.tile([112, 2, 448], BF, tag="e", name="e")
                            if _exp_eng(hg, jt, p2) == "A":
                                nc.scalar.activation(
                                    e4[:JT, :, :], s4s[p2][:JT, :, :IC_F], AF.Exp, scale=SCALE
                                )
                            else:
                                nc.vector.tensor_scalar(
                                    e4[:JT, :, :].bitcast(I16),
                                    s4s[p2][:JT, :, :IC_F],
                                    A_EXP,
                                    B_EXP,
                                    ALU.mult,
                                    ALU.add,
                                )
                            e4p[p2] = e4
                        if jt + 1 < N_JT:
                            s4s = [s_pair(jt + 1, 0), None]
                        for p2 in range(2):
                            for hh in (2 * p2, 2 * p2 + 1):
                                tile_, base = o_ps4[hh]
                                nc.tensor.matmul(
                                    tile_[base:base + 64, :],
                                    lhsT=(v_tm[:JT, jt, hg * 4 + hh, :]),
                                    rhs=(e4p[p2][:JT, hh % 2, :]),
                                    start=(jt == 0),
                                    stop=(jt == N_JT - 1),
                                    tile_position=(0, base),
                                    skip_group_check=True,
                                )
                        if jt + 1 < N_JT:
                            s4s[1] = s_pair(jt + 1, 1)

                    # normalize: recip whole psum tiles (d sits replicated in
                    # rows base+32:base+64), then 4 mixed-space multiplies
                    r_f = [rp.tile([128, IC_F], FP, tag=f"rf{t}", name=f"rf{t}") for t in range(2)]
                    nc.vector.reciprocal_approx_fast(r_f[0][:], o_psA[:])
                    nc.vector.reciprocal_approx_fast(r_f[1][:], o_psB[:])
                    for hh in range(4):
                        tile_, base = o_ps4[hh]
                        r_f_t = r_f[0] if hh < 2 else r_f[1]
                        nc.vector.tensor_mul(
                            o_cm[hg][hh * 32:(hh + 1) * 32, ic * IC_F:(ic + 1) * IC_F],
                            tile_[base:base + 32, :],
                            r_f_t[base + 32:base + 64, :],
                        )

                    # after hg0: pw for chunk ic+1; dw chunk ic+2 split so the
                    # DVE queue never has a long burst ahead of exp tiles
                    if hg == 0 and ic + 1 < N_IC:
                        pw_cm("q", q_cm, dwq, (ic + 1) * IC_F, (ic + 2) * IC_F,
                              ((ic + 1) * IC_CH, (ic + 2) * IC_CH), HH)
                        if ic + 2 < N_IC:
                            dw_q(dwq, (ic + 2) * IC_CH, (ic + 3) * IC_CH, cbis=(0,))
                    if hg == 1 and ic + 2 < N_IC:
                        dw_q(dwq, (ic + 2) * IC_CH, (ic + 3) * IC_CH, cbis=(1,))

                # proj for all fully-covered 128-token blocks
                while ti_ready < n_tt and min(ti_ready * 128, N - 128) + 128 <= (ic + 1) * IC_F:
                    proj_block(min(ti_ready * 128, N - 128))
                    ti_ready += 1

    nc.compile()
    return nc


def _fold_common(inp):
    common = {}
    for p in ("q", "k", "v"):
        scale = inp[f"{p}_bn_g"] / np.sqrt(inp[f"{p}_bn_v"] + EPS)
        shift = inp[f"{p}_bn_b"] - inp[f"{p}_bn_m"] * scale
        const = (
            inp[f"{p}_pw_w"] @ (scale * inp[f"{p}_dw_b"] + shift) + inp[f"{p}_pw_b"]
        ).astype(np.float32)
        if p == "v":
            common["v_const_r1"] = const.reshape(1, C).astype(ml_dtypes.bfloat16)
        else:
            common[f"{p}_const"] = const.reshape(C, 1)
        dw9 = (inp[f"{p}_dw_w"].reshape(C, 9) * scale[:, None]).astype(np.float32)
        common[f"{p}_dww"] = np.ascontiguousarray(
            dw9.reshape(2, 128, 9).transpose(1, 0, 2)
        )
        common[f"{p}_pww"] = np.ascontiguousarray(
            inp[f"{p}_pw_w"].T.reshape(2, 128, C).transpose(1, 0, 2)
        ).astype(ml_dtypes.bfloat16)
    common["proj_wT"] = np.ascontiguousarray(
        inp["proj_w"].T.reshape(2, 128, C)
    ).astype(ml_dtypes.bfloat16)
    common["proj_brep"] = np.ascontiguousarray(
        np.broadcast_to(inp["proj_b"].reshape(1, C), (128, C))
    ).astype(np.float32)
    return common


def prepare_x(xb):
    # xb: (3136, 256) f32 -> padded channel-major bf16 (128, 2, 58, 58)
    xt = xb.T.reshape(C, HH, HH)
    xp = np.zeros((C, PADW, PADW), np.float32)
    xp[:, 1:57, 1:57] = xt
    return np.ascontiguousarray(
        xp.reshape(2, 128, PADW, PADW).transpose(1, 0, 2, 3)
    ).astype(ml_dtypes.bfloat16)


def prepare_in_maps(inp):
    common = _fold_common(inp)
    x = inp["x"].astype(np.float32)
    return [dict(common, x_pad=prepare_x(x[b])) for b in range(x.shape[0])]


def kernel(**inputs):
    inp = {k: np.asarray(v) for k, v in inputs.items()}

    if "nc" not in _CACHED:
        _CACHED["nc"] = _build_nc()
    nc = _CACHED["nc"]

    in_maps = prepare_in_maps(inp)
    res = run_bass_kernel_spmd(nc, in_maps, list(range(len(in_maps))))
    out = np.stack([res.results[b]["out"] for b in range(len(in_maps))], axis=0)
    return out.astype(np.float32)


# revision 34
# speedup vs baseline: 1.1861x; 1.1861x over previous
"""Trainium2 Bass kernel for nn_Attention_81776177315877.

Separable-conv attention block (CMT/PVT style):
  x (B=8, 3136, 256) -> q/k/v = sepconv(dw3x3+BN+pw1x1, k/v stride 2)
  -> 8-head attention (d=32) -> proj.

Sharding: data-parallel over batch, core b <- batch b. No collectives.

v6 design (per core):
  - depthwise 3x3 on DVE (9 chained per-partition-scalar MACs, bf16).
    All tap windows are 4-byte aligned via host-side shifted/split copies
    of x (x_pad/x_psh for q; x_e/x_o/x_es for stride-2 k/v) so the DVE
    2x 16-bit mode can engage.
  - pointwise is a true K=256 matmul; v's pointwise runs data-as-weights
    so it directly yields token-major v tiles (no PE transposes).
  - v tiles carry a 33rd column of ones: the O-wave (M=33) computes the
    softmax denominator as a by-product, eliminating the D-wave entirely
    (1/3 of the attention matmul columns).
  - o/d normalize: DVE reciprocal on the psum tile, d-rows extracted by
    ACT copies, GpSimd partition_broadcast replicates, DVE multiplies.
  - rank-1 (K=1) PE matmuls preload biases into PSUM accumulation, so
    PSUM->SBUF staging is a plain ACT copy.
  - exp split ACT (native) / DVE (Schraudolph bitcast), S runs one jt
    ahead.
"""

import sys

sys.path.insert(0, "/opt/trn_rl_repo")

import numpy as np
import ml_dtypes

import concourse.bass as bass
import concourse.bacc as bacc
import concourse.mybir as mybir
import concourse.tile as tile
from concourse.bass_utils import run_bass_kernel_spmd

FP = mybir.dt.float32
BF = mybir.dt.bfloat16
I16 = mybir.dt.int16
AF = mybir.ActivationFunctionType
ALU = mybir.AluOpType

C = 256
HEADS = 8
D = 32
HH = 56
N = HH * HH          # 3136 query tokens
HK = 28
NK = HK * HK         # 784 key tokens
PADW = HH + 2        # 58
KW = 30              # padded width of x_e/x_o/x_es and dwk/dwv rows
EPS = 1e-5
SCALE = D ** -0.5

IC_CH = 8            # query rows per chunk -> 448 free
IC_F = IC_CH * HH    # 448
N_IC = HH // IC_CH   # 7
JT = 112             # key tile (partitions) for attention
N_JT = NK // JT      # 7
VROWS = JT // HK     # 4 dw rows of v per jt chunk

# Schraudolph fast exp in bf16 bits: i16 = A_EXP * S + B_EXP, bits -> bf16
A_EXP = 184.6649652 * SCALE
B_EXP = 16250.5

_CACHED = {}


def _exp_eng(hg, jt, p2):
    """Exp tile engine: 'A' (ACT native) or 'D' (DVE Schraudolph)."""
    return "A"


def _build_nc():
    nc = bacc.Bacc("TRN2", target_bir_lowering=False, debug=False, num_devices=8)

    xpad_d = nc.dram_tensor("x_pad", [128, 2, PADW, PADW], BF, kind="ExternalInput")
    dww_d = {}
    pww_d = {}
    for p in ("q", "k", "v"):
        dww_d[p] = nc.dram_tensor(f"{p}_dww", [128, 2, 9], FP, kind="ExternalInput")
        pww_d[p] = nc.dram_tensor(f"{p}_pww", [128, 2, C], BF, kind="ExternalInput")
    const_d = {}
    for p in ("q", "k"):
        const_d[p] = nc.dram_tensor(f"{p}_const", [C, 1], FP, kind="ExternalInput")
    const_d["v"] = nc.dram_tensor("v_const_r1", [1, C], BF, kind="ExternalInput")
    pb_d = nc.dram_tensor("proj_b_r1", [1, C], BF, kind="ExternalInput")
    pwT_d = nc.dram_tensor("proj_wT", [2, 128, C], BF, kind="ExternalInput")
    out_d = nc.dram_tensor("out", [N, C], FP, kind="ExternalOutput")

    with tile.TileContext(nc) as tc:
        with (
            tc.tile_pool(name="persist", bufs=1) as pp,
            tc.tile_pool(name="ep", bufs=8) as ep,
            tc.tile_pool(name="rp", bufs=2) as rp,
            tc.tile_pool(name="op", bufs=3) as otp,
            tc.tile_pool(name="psS", bufs=3, space="PSUM") as psS,
            tc.tile_pool(name="psA", bufs=1, space="PSUM") as psA,
            tc.tile_pool(name="psB", bufs=1, space="PSUM") as psB,
        ):
            # ---- input / weight loads ----
            dww = {}
            pww = {}
            for p in ("k", "v", "q"):
                dww[p] = pp.tile([128, 2, 9], FP, tag=f"dww{p}", name=f"dww{p}")
                pww[p] = pp.tile([128, 2, C], BF, tag=f"pww{p}", name=f"pww{p}")
            nc.sync.dma_start(dww["k"][:], dww_d["k"][:])
            nc.sync.dma_start(dww["v"][:], dww_d["v"][:])
            nc.sync.dma_start(dww["q"][:], dww_d["q"][:])
            x_pad = pp.tile([128, 2, PADW, PADW], BF, tag="xpad", name="xpad")
            nc.scalar.dma_start(x_pad[:, 0], xpad_d[:, 0])
            nc.scalar.dma_start(x_pad[:, 1], xpad_d[:, 1])
            for p in ("k", "v", "q"):
                nc.gpsimd.dma_start(pww[p][:], pww_d[p][:])
            consts = {}
            for p in ("q", "k"):
                consts[p] = [
                    pp.tile([128, 1], FP, tag=f"const_{p}{cb}", name=f"const_{p}{cb}")
                    for cb in range(2)
                ]
                for cb in range(2):
                    nc.gpsimd.dma_start(consts[p][cb][:], const_d[p][cb * 128:(cb + 1) * 128, :])
            consts["v"] = pp.tile([1, C], BF, tag="const_v", name="const_v")
            nc.gpsimd.dma_start(consts["v"][:], const_d["v"][:])
            pb_r1 = pp.tile([1, C], BF, tag="pbr1", name="pbr1")
            nc.gpsimd.dma_start(pb_r1[:], pb_d[:])
            pwT = [pp.tile([128, C], BF, tag=f"pwT{cb}", name=f"pwT{cb}") for cb in range(2)]
            for cb in range(2):
                nc.gpsimd.dma_start(pwT[cb][:], pwT_d[cb, :, :])
            ones_r1 = pp.tile([1, 512], BF, tag="ones_r1", name="ones_r1")
            nc.vector.memset(ones_r1[:], 1.0)

            # stride-2 view of x_pad: (row, parity, col, parity)
            xp2 = x_pad[:].rearrange("p b (ho a) (wv c) -> p b ho a wv c", a=2, c=2)

            # ---- depthwise helpers (all DVE) ----
            def dw_kv(p, dst, ro0, ro1):
                """stride-2 depthwise rows [ro0, ro1): out[ro,wo] =
                sum_t w[t] x_pad[1+2ro+dh, 1+2wo+dwd]."""
                nr = ro1 - ro0
                for cbi in range(2):
                    first = True
                    for tap in range(9):
                        dh, dwd = tap // 3 - 1, tap % 3 - 1
                        r0 = 1 + 2 * ro0 + dh
                        c0 = 1 + dwd
                        src = xp2[:, cbi, r0 // 2: r0 // 2 + nr, r0 % 2,
                                  c0 // 2: c0 // 2 + HK, c0 % 2]
                        w = dww[p][:, cbi, tap:tap + 1]
                        dd = dst[:, cbi, ro0:ro1, :]
                        if first:
                            nc.vector.tensor_scalar(dd, src, w, None, ALU.mult)
                            first = False
                        else:
                            nc.vector.scalar_tensor_tensor(dd, src, w, dd, ALU.mult, ALU.add)

            def dw_q(dst, ro0, ro1, cbis=(0, 1)):
                """stride-1 depthwise rows [ro0, ro1)."""
                for cbi in cbis:
                    first = True
                    for tap in range(9):
                        dh, dwd = tap // 3 - 1, tap % 3 - 1
                        src = x_pad[:, cbi, 1 + ro0 + dh:1 + ro1 + dh, 1 + dwd:1 + dwd + HH]
                        w = dww["q"][:, cbi, tap:tap + 1]
                        dd = dst[:, cbi, ro0:ro1, :]
                        if first:
                            nc.vector.tensor_scalar(dd, src, w, None, ALU.mult)
                            first = False
                        else:
                            nc.vector.scalar_tensor_tensor(dd, src, w, dd, ALU.mult, ALU.add)

            dwk = pp.tile([128, 2, HK, HK], BF, tag="dwk", name="dwk")
            dwv = pp.tile([128, 2, HK, HK], BF, tag="dwv", name="dwv")
            dwq = pp.tile([128, 2, HH, HH], BF, tag="dwq", name="dwq")

            k_cm = [pp.tile([128, NK], BF, tag=f"kcm{cb}", name=f"kcm{cb}") for cb in range(2)]
            q_cm = [pp.tile([128, N], BF, tag=f"qcm{cb}", name=f"qcm{cb}") for cb in range(2)]
            o_cm = [pp.tile([128, N], BF, tag=f"ocm{cb}", name=f"ocm{cb}") for cb in range(2)]
            # v token-major with 32 ones columns per head: the O-wave (M=64)
            # emits the softmax denominator pre-replicated in rows 32:64
            v_tm = pp.tile([128, N_JT, 8, 64], BF, tag="vtm", name="vtm")
            nc.vector.memset(v_tm[:, :, :, 32:64], 1.0)

            # ---- pointwise helpers (PE) ----
            def pw_cm(p, dst_tiles, src, t0, t1, src_rows, width):
                """channel-major pointwise with rank-1 bias preload.
                Token range [t0, t1) in units of `width`-wide rows."""
                fsz = t1 - t0
                for half in range(2):
                    cps = (psA if half == 0 else psB).tile(
                        [128, 448], FP, tag=f"ps{'AB'[half]}", name="cps"
                    )
                    for cbi in range(2):
                        nc.tensor.matmul(
                            cps[:, :fsz],
                            lhsT=(pww[p][:, cbi, half * 128:(half + 1) * 128]),
                            rhs=(src[:, cbi, src_rows[0]:src_rows[1], 0:width]),
                            start=(cbi == 0),
                            stop=(cbi == 1),
                        )
                    nc.scalar.activation(
                        dst_tiles[half][:, t0:t1], cps[:, :fsz],
                        AF.Identity, bias=consts[p][half],
                    )

            def pw_v(jt):
                """token-major pointwise for v -> v_tm[0:112, jt, :, 0:32]."""
                vps = (psA if jt % 2 == 0 else psB).tile(
                    [128, 448], FP, tag=f"ps{'AB'[jt % 2]}", name="vps"
                )
                nc.tensor.matmul(
                    vps[:JT, :C],
                    lhsT=(ones_r1[:, :JT]),
                    rhs=(consts["v"][:, :]),
                    start=True,
                    stop=False,
                )
                dv = dwv[:].rearrange("p b r w -> p b (r w)")
                for cbi in range(2):
                    nc.tensor.matmul(
                        vps[:JT, :C],
                        lhsT=(dv[:, cbi, jt * JT:(jt + 1) * JT]),
                        rhs=(pww["v"][:, cbi, :]),
                        start=False,
                        stop=(cbi == 1),
                    )
                nc.scalar.activation(v_tm[:JT, jt, :, 0:32], vps[:JT, :C], AF.Copy)

            # ---- startup (ordered so attention can begin after k-half + q0) ----
            dw_kv("k", dwk, 0, 14)
            pw_cm("k", k_cm, dwk, 0, 14 * HK, (0, 14), HK)
            dw_q(dwq, 0, IC_CH)
            pw_cm("q", q_cm, dwq, 0, IC_F, (0, IC_CH), HH)
            dw_kv("k", dwk, 14, HK)
            pw_cm("k", k_cm, dwk, 14 * HK, NK, (14, HK), HK)
            dw_kv("v", dwv, 0, HK)
            for jt in range(N_JT):
                pw_v(jt)
            dw_q(dwq, IC_CH, 2 * IC_CH)

            # ---- main loop ----
            n_tt = (N + 127) // 128  # 25 output token blocks
            ti_ready = 0

            def proj_block(st):
                pps = psB.tile([128, 448], FP, tag="psB", name="pj")
                nc.tensor.matmul(
                    pps[:, :C],
                    lhsT=(ones_r1[:, :128]),
                    rhs=(pb_r1[:, :]),
                    start=True,
                    stop=False,
                )
                for cb in range(2):
                    nc.tensor.matmul(
                        pps[:, :C],
                        lhsT=(o_cm[cb][:, st:st + 128]),
                        rhs=(pwT[cb][:, :]),
                        start=False,
                        stop=(cb == 1),
                    )
                ot = otp.tile([128, C], FP, tag="ot", name="ot")
                nc.scalar.activation(ot[:], pps[:, :C], AF.Copy)
                nc.sync.dma_start(out_d[st:st + 128, :], ot[:])

            for ic in range(N_IC):
                for hg in range(2):
                    o_psA = psA.tile([128, 448], FP, tag="psA", name="oA")
                    o_psB = psB.tile([128, 448], FP, tag="psB", name="oB")
                    o_ps4 = {0: (o_psA, 0), 1: (o_psA, 64), 2: (o_psB, 0), 3: (o_psB, 64)}

                    def s_pair(jt, p2):
                        s4p = psS.tile([112, 2, 512], FP, tag="s4", name="s4")
                        for hh in (2 * p2, 2 * p2 + 1):
                            nc.tensor.matmul(
                                s4p[:JT, hh % 2, :IC_F],
                                lhsT=(k_cm[hg][hh * 32:(hh + 1) * 32, jt * JT:(jt + 1) * JT]),
                                rhs=(q_cm[hg][hh * 32:(hh + 1) * 32, ic * IC_F:(ic + 1) * IC_F]),
                                start=True,
                                stop=True,
                                tile_position=(32 * hh, 0),
                                skip_group_check=True,
                            )
                        return s4p

                    s4s = [s_pair(0, 0), s_pair(0, 1)]
                    for jt in range(N_JT):
                        e4p = [None, None]
                        for p2 in range(2):
                            e4 = ep.tile([112, 2, 448], BF, tag="e", name="e")
                            if _exp_eng(hg, jt, p2) == "A":
                                nc.scalar.activation(
                                    e4[:JT, :, :], s4s[p2][:JT, :, :IC_F], AF.Exp, scale=SCALE
                                )
                            else:
                                nc.vector.tensor_scalar(
                                    e4[:JT, :, :].bitcast(I16),
                                    s4s[p2][:JT, :, :IC_F],
                                    A_EXP,
                                    B_EXP,
                                    ALU.mult,
                                    ALU.add,
                                )
                            e4p[p2] = e4
                        if jt + 1 < N_JT:
                            s4s = [s_pair(jt + 1, 0), None]
                        for p2 in range(2):
                            for hh in (2 * p2, 2 * p2 + 1):
                                tile_, base = o_ps4[hh]
                                nc.tensor.matmul(
                                    tile_[base:base + 64, :],
                                    lhsT=(v_tm[:JT, jt, hg * 4 + hh, :]),
                                    rhs=(e4p[p2][:JT, hh % 2, :]),
                                    start=(jt == 0),
                                    stop=(jt == N_JT - 1),
                                    tile_position=(0, base),
                                    skip_group_check=True,
                                )
                        if jt + 1 < N_JT:
                            s4s[1] = s_pair(jt + 1, 1)

                    # normalize: recip whole psum tiles (d sits replicated in
                    # rows base+32:base+64), then 4 mixed-space multiplies
                    r_f = [rp.tile([128, IC_F], FP, tag=f"rf{t}", name=f"rf{t}") for t in range(2)]
                    nc.vector.reciprocal_approx_fast(r_f[0][:], o_psA[:])
                    nc.vector.reciprocal_approx_fast(r_f[1][:], o_psB[:])
                    for hh in range(4):
                        tile_, base = o_ps4[hh]
                        r_f_t = r_f[0] if hh < 2 else r_f[1]
                        nc.vector.tensor_mul(
                            o_cm[hg][hh * 32:(hh + 1) * 32, ic * IC_F:(ic + 1) * IC_F],
                            tile_[base:base + 32, :],
                            r_f_t[base + 32:base + 64, :],
                        )

                    # after hg0: pw for chunk ic+1; dw chunk ic+2 split so the
                    # DVE queue never has a long burst ahead of exp tiles
                    if hg == 0 and ic + 1 < N_IC:
                        pw_cm("q", q_cm, dwq, (ic + 1) * IC_F, (ic + 2) * IC_F,
                              ((ic + 1) * IC_CH, (ic + 2) * IC_CH), HH)
                        if ic + 2 < N_IC:
                            dw_q(dwq, (ic + 2) * IC_CH, (ic + 3) * IC_CH, cbis=(0,))
                    if hg == 1 and ic + 2 < N_IC:
                        dw_q(dwq, (ic + 2) * IC_CH, (ic + 3) * IC_CH, cbis=(1,))

                # proj for all fully-covered 128-token blocks
                while ti_ready < n_tt and min(ti_ready * 128, N - 128) + 128 <= (ic + 1) * IC_F:
                    proj_block(min(ti_ready * 128, N - 128))
                    ti_ready += 1

    nc.compile()
    return nc


def _fold_common(inp):
    common = {}
    for p in ("q", "k", "v"):
        scale = inp[f"{p}_bn_g"] / np.sqrt(inp[f"{p}_bn_v"] + EPS)
        shift = inp[f"{p}_bn_b"] - inp[f"{p}_bn_m"] * scale
        const = (
            inp[f"{p}_pw_w"] @ (scale * inp[f"{p}_dw_b"] + shift) + inp[f"{p}_pw_b"]
        ).astype(np.float32)
        if p == "v":
            common["v_const_r1"] = const.reshape(1, C).astype(ml_dtypes.bfloat16)
        else:
            common[f"{p}_const"] = const.reshape(C, 1)
        dw9 = (inp[f"{p}_dw_w"].reshape(C, 9) * scale[:, None]).astype(np.float32)
        common[f"{p}_dww"] = np.ascontiguousarray(
            dw9.reshape(2, 128, 9).transpose(1, 0, 2)
        )
        common[f"{p}_pww"] = np.ascontiguousarray(
            inp[f"{p}_pw_w"].T.reshape(2, 128, C).transpose(1, 0, 2)
        ).astype(ml_dtypes.bfloat16)
    common["proj_wT"] = np.ascontiguousarray(
        inp["proj_w"].T.reshape(2, 128, C)
    ).astype(ml_dtypes.bfloat16)
    common["proj_b_r1"] = inp["proj_b"].reshape(1, C).astype(ml_dtypes.bfloat16)
    return common


def prepare_x(xb):
    # xb: (3136, 256) f32 -> padded channel-major bf16 (128, 2, 58, 58)
    xt = xb.T.reshape(C, HH, HH)
    xp = np.zeros((C, PADW, PADW), np.float32)
    xp[:, 1:57, 1:57] = xt
    return np.ascontiguousarray(
        xp.reshape(2, 128, PADW, PADW).transpose(1, 0, 2, 3)
    ).astype(ml_dtypes.bfloat16)


def prepare_in_maps(inp):
    common = _fold_common(inp)
    x = inp["x"].astype(np.float32)
    return [dict(common, x_pad=prepare_x(x[b])) for b in range(x.shape[0])]


def kernel(**inputs):
    inp = {k: np.asarray(v) for k, v in inputs.items()}

    if "nc" not in _CACHED:
        _CACHED["nc"] = _build_nc()
    nc = _CACHED["nc"]

    in_maps = prepare_in_maps(inp)
    res = run_bass_kernel_spmd(nc, in_maps, list(range(len(in_maps))))
    out = np.stack([res.results[b]["out"] for b in range(len(in_maps))], axis=0)
    return out.astype(np.float32)


# revision 35
# speedup vs baseline: 1.1953x; 1.0077x over previous
"""Trainium2 Bass kernel for nn_Attention_81776177315877.

Separable-conv attention block (CMT/PVT style):
  x (B=8, 3136, 256) -> q/k/v = sepconv(dw3x3+BN+pw1x1, k/v stride 2)
  -> 8-head attention (d=32) -> proj.

Sharding: data-parallel over batch, core b <- batch b. No collectives.

v6 design (per core):
  - depthwise 3x3 on DVE (9 chained per-partition-scalar MACs, bf16).
    All tap windows are 4-byte aligned via host-side shifted/split copies
    of x (x_pad/x_psh for q; x_e/x_o/x_es for stride-2 k/v) so the DVE
    2x 16-bit mode can engage.
  - pointwise is a true K=256 matmul; v's pointwise runs data-as-weights
    so it directly yields token-major v tiles (no PE transposes).
  - v tiles carry a 33rd column of ones: the O-wave (M=33) computes the
    softmax denominator as a by-product, eliminating the D-wave entirely
    (1/3 of the attention matmul columns).
  - o/d normalize: DVE reciprocal on the psum tile, d-rows extracted by
    ACT copies, GpSimd partition_broadcast replicates, DVE multiplies.
  - rank-1 (K=1) PE matmuls preload biases into PSUM accumulation, so
    PSUM->SBUF staging is a plain ACT copy.
  - exp split ACT (native) / DVE (Schraudolph bitcast), S runs one jt
    ahead.
"""

import sys

sys.path.insert(0, "/opt/trn_rl_repo")

import numpy as np
import ml_dtypes

import concourse.bass as bass
import concourse.bacc as bacc
import concourse.mybir as mybir
import concourse.tile as tile
from concourse.bass_utils import run_bass_kernel_spmd

FP = mybir.dt.float32
BF = mybir.dt.bfloat16
I16 = mybir.dt.int16
AF = mybir.ActivationFunctionType
ALU = mybir.AluOpType

C = 256
HEADS = 8
D = 32
HH = 56
N = HH * HH          # 3136 query tokens
HK = 28
NK = HK * HK         # 784 key tokens
PADW = HH + 2        # 58
KW = 30              # padded width of x_e/x_o/x_es and dwk/dwv rows
EPS = 1e-5
SCALE = D ** -0.5

IC_CH = 8            # query rows per chunk -> 448 free
IC_F = IC_CH * HH    # 448
N_IC = HH // IC_CH   # 7
JT = 112             # key tile (partitions) for attention
N_JT = NK // JT      # 7
VROWS = JT // HK     # 4 dw rows of v per jt chunk

# Schraudolph fast exp in bf16 bits: i16 = A_EXP * S + B_EXP, bits -> bf16
A_EXP = 184.6649652 * SCALE
B_EXP = 16250.5

_CACHED = {}


def _exp_eng(hg, jt, p2):
    """Exp tile engine: 'A' (ACT native) or 'D' (DVE Schraudolph)."""
    return "A"


def _build_nc():
    nc = bacc.Bacc("TRN2", target_bir_lowering=False, debug=False, num_devices=8)

    xpad_d = nc.dram_tensor("x_pad", [128, 2, PADW, PADW], BF, kind="ExternalInput")
    dww_d = {}
    pww_d = {}
    for p in ("q", "k", "v"):
        dww_d[p] = nc.dram_tensor(f"{p}_dww", [128, 2, 9], FP, kind="ExternalInput")
        pww_d[p] = nc.dram_tensor(f"{p}_pww", [128, 2, C], BF, kind="ExternalInput")
    const_d = {}
    for p in ("q", "k"):
        const_d[p] = nc.dram_tensor(f"{p}_const", [C, 1], FP, kind="ExternalInput")
    const_d["v"] = nc.dram_tensor("v_const_r1", [1, C], BF, kind="ExternalInput")
    pb_d = nc.dram_tensor("proj_brep", [128, C], FP, kind="ExternalInput")
    pwT_d = nc.dram_tensor("proj_wT", [2, 128, C], BF, kind="ExternalInput")
    out_d = nc.dram_tensor("out", [N, C], FP, kind="ExternalOutput")

    with tile.TileContext(nc) as tc:
        with (
            tc.tile_pool(name="persist", bufs=1) as pp,
            tc.tile_pool(name="ep", bufs=8) as ep,
            tc.tile_pool(name="rp", bufs=2) as rp,
            tc.tile_pool(name="op", bufs=3) as otp,
            tc.tile_pool(name="psS", bufs=3, space="PSUM") as psS,
            tc.tile_pool(name="psA", bufs=1, space="PSUM") as psA,
            tc.tile_pool(name="psB", bufs=1, space="PSUM") as psB,
        ):
            # ---- input / weight loads ----
            dww = {}
            pww = {}
            for p in ("k", "v", "q"):
                dww[p] = pp.tile([128, 2, 9], FP, tag=f"dww{p}", name=f"dww{p}")
                pww[p] = pp.tile([128, 2, C], BF, tag=f"pww{p}", name=f"pww{p}")
            nc.sync.dma_start(dww["k"][:], dww_d["k"][:])
            nc.sync.dma_start(dww["v"][:], dww_d["v"][:])
            nc.sync.dma_start(dww["q"][:], dww_d["q"][:])
            x_pad = pp.tile([128, 2, PADW, PADW], BF, tag="xpad", name="xpad")
            nc.scalar.dma_start(x_pad[:, 0], xpad_d[:, 0])
            nc.scalar.dma_start(x_pad[:, 1], xpad_d[:, 1])
            for p in ("k", "v", "q"):
                nc.gpsimd.dma_start(pww[p][:], pww_d[p][:])
            consts = {}
            for p in ("q", "k"):
                consts[p] = [
                    pp.tile([128, 1], FP, tag=f"const_{p}{cb}", name=f"const_{p}{cb}")
                    for cb in range(2)
                ]
                for cb in range(2):
                    nc.gpsimd.dma_start(consts[p][cb][:], const_d[p][cb * 128:(cb + 1) * 128, :])
            consts["v"] = pp.tile([1, C], BF, tag="const_v", name="const_v")
            nc.gpsimd.dma_start(consts["v"][:], const_d["v"][:])
            pb_rep = pp.tile([128, C], FP, tag="pbrep", name="pbrep")
            nc.gpsimd.dma_start(pb_rep[:], pb_d[:])
            pwT = [pp.tile([128, C], BF, tag=f"pwT{cb}", name=f"pwT{cb}") for cb in range(2)]
            for cb in range(2):
                nc.gpsimd.dma_start(pwT[cb][:], pwT_d[cb, :, :])
            ones_r1 = pp.tile([1, 512], BF, tag="ones_r1", name="ones_r1")
            nc.vector.memset(ones_r1[:], 1.0)

            # stride-2 view of x_pad: (row, parity, col, parity)
            xp2 = x_pad[:].rearrange("p b (ho a) (wv c) -> p b ho a wv c", a=2, c=2)

            # ---- depthwise helpers (all DVE) ----
            def dw_kv(p, dst, ro0, ro1):
                """stride-2 depthwise rows [ro0, ro1): out[ro,wo] =
                sum_t w[t] x_pad[1+2ro+dh, 1+2wo+dwd]."""
                nr = ro1 - ro0
                for cbi in range(2):
                    first = True
                    for tap in range(9):
                        dh, dwd = tap // 3 - 1, tap % 3 - 1
                        r0 = 1 + 2 * ro0 + dh
                        c0 = 1 + dwd
                        src = xp2[:, cbi, r0 // 2: r0 // 2 + nr, r0 % 2,
                                  c0 // 2: c0 // 2 + HK, c0 % 2]
                        w = dww[p][:, cbi, tap:tap + 1]
                        dd = dst[:, cbi, ro0:ro1, :]
                        if first:
                            nc.vector.tensor_scalar(dd, src, w, None, ALU.mult)
                            first = False
                        else:
                            nc.vector.scalar_tensor_tensor(dd, src, w, dd, ALU.mult, ALU.add)

            def dw_q(dst, ro0, ro1, cbis=(0, 1)):
                """stride-1 depthwise rows [ro0, ro1)."""
                for cbi in cbis:
                    first = True
                    for tap in range(9):
                        dh, dwd = tap // 3 - 1, tap % 3 - 1
                        src = x_pad[:, cbi, 1 + ro0 + dh:1 + ro1 + dh, 1 + dwd:1 + dwd + HH]
                        w = dww["q"][:, cbi, tap:tap + 1]
                        dd = dst[:, cbi, ro0:ro1, :]
                        if first:
                            nc.vector.tensor_scalar(dd, src, w, None, ALU.mult)
                            first = False
                        else:
                            nc.vector.scalar_tensor_tensor(dd, src, w, dd, ALU.mult, ALU.add)

            dwk = pp.tile([128, 2, HK, HK], BF, tag="dwk", name="dwk")
            dwv = pp.tile([128, 2, HK, HK], BF, tag="dwv", name="dwv")
            dwq = pp.tile([128, 2, HH, HH], BF, tag="dwq", name="dwq")

            k_cm = [pp.tile([128, NK], BF, tag=f"kcm{cb}", name=f"kcm{cb}") for cb in range(2)]
            q_cm = [pp.tile([128, N], BF, tag=f"qcm{cb}", name=f"qcm{cb}") for cb in range(2)]
            o_cm = [pp.tile([128, N], BF, tag=f"ocm{cb}", name=f"ocm{cb}") for cb in range(2)]
            # v token-major with 32 ones columns per head: the O-wave (M=64)
            # emits the softmax denominator pre-replicated in rows 32:64
            v_tm = pp.tile([128, N_JT, 8, 64], BF, tag="vtm", name="vtm")
            nc.vector.memset(v_tm[:, :, :, 32:64], 1.0)

            # ---- pointwise helpers (PE) ----
            def pw_cm(p, dst_tiles, src, t0, t1, src_rows, width):
                """channel-major pointwise with rank-1 bias preload.
                Token range [t0, t1) in units of `width`-wide rows."""
                fsz = t1 - t0
                for half in range(2):
                    cps = (psA if half == 0 else psB).tile(
                        [128, 448], FP, tag=f"ps{'AB'[half]}", name="cps"
                    )
                    for cbi in range(2):
                        nc.tensor.matmul(
                            cps[:, :fsz],
                            lhsT=(pww[p][:, cbi, half * 128:(half + 1) * 128]),
                            rhs=(src[:, cbi, src_rows[0]:src_rows[1], 0:width]),
                            start=(cbi == 0),
                            stop=(cbi == 1),
                        )
                    nc.vector.tensor_scalar(
                        dst_tiles[half][:, t0:t1], cps[:, :fsz],
                        consts[p][half], None, ALU.add,
                    )

            def pw_v(jt):
                """token-major pointwise for v -> v_tm[0:112, jt, :, 0:32]."""
                vps = (psA if jt % 2 == 0 else psB).tile(
                    [128, 448], FP, tag=f"ps{'AB'[jt % 2]}", name="vps"
                )
                nc.tensor.matmul(
                    vps[:JT, :C],
                    lhsT=(ones_r1[:, :JT]),
                    rhs=(consts["v"][:, :]),
                    start=True,
                    stop=False,
                )
                dv = dwv[:].rearrange("p b r w -> p b (r w)")
                for cbi in range(2):
                    nc.tensor.matmul(
                        vps[:JT, :C],
                        lhsT=(dv[:, cbi, jt * JT:(jt + 1) * JT]),
                        rhs=(pww["v"][:, cbi, :]),
                        start=False,
                        stop=(cbi == 1),
                    )
                nc.scalar.activation(v_tm[:JT, jt, :, 0:32], vps[:JT, :C], AF.Copy)

            # ---- startup (ordered so attention can begin after k-half + q0) ----
            dw_kv("k", dwk, 0, 14)
            pw_cm("k", k_cm, dwk, 0, 14 * HK, (0, 14), HK)
            dw_q(dwq, 0, IC_CH)
            pw_cm("q", q_cm, dwq, 0, IC_F, (0, IC_CH), HH)
            dw_kv("k", dwk, 14, HK)
            pw_cm("k", k_cm, dwk, 14 * HK, NK, (14, HK), HK)
            dw_kv("v", dwv, 0, HK)
            for jt in range(N_JT):
                pw_v(jt)
            dw_q(dwq, IC_CH, 2 * IC_CH)

            # ---- main loop ----
            n_tt = (N + 127) // 128  # 25 output token blocks
            ti_ready = 0

            def proj_block(st):
                pps = psB.tile([128, 448], FP, tag="psB", name="pj")
                for cb in range(2):
                    nc.tensor.matmul(
                        pps[:, :C],
                        lhsT=(o_cm[cb][:, st:st + 128]),
                        rhs=(pwT[cb][:, :]),
                        start=(cb == 0),
                        stop=(cb == 1),
                    )
                ot = otp.tile([128, C], FP, tag="ot", name="ot")
                nc.vector.scalar_tensor_tensor(
                    ot[:], pps[:, :C], 1.0, pb_rep[:], ALU.mult, ALU.add
                )
                nc.sync.dma_start(out_d[st:st + 128, :], ot[:])

            for ic in range(N_IC):
                for hg in range(2):
                    o_psA = psA.tile([128, 448], FP, tag="psA", name="oA")
                    o_psB = psB.tile([128, 448], FP, tag="psB", name="oB")
                    o_ps4 = {0: (o_psA, 0), 1: (o_psA, 64), 2: (o_psB, 0), 3: (o_psB, 64)}

                    def s_pair(jt, p2):
                        s4p = psS.tile([112, 2, 512], FP, tag="s4", name="s4")
                        for hh in (2 * p2, 2 * p2 + 1):
                            nc.tensor.matmul(
                                s4p[:JT, hh % 2, :IC_F],
                                lhsT=(k_cm[hg][hh * 32:(hh + 1) * 32, jt * JT:(jt + 1) * JT]),
                                rhs=(q_cm[hg][hh * 32:(hh + 1) * 32, ic * IC_F:(ic + 1) * IC_F]),
                                start=True,
                                stop=True,
                                tile_position=(32 * hh, 0),
                                skip_group_check=True,
                            )
                        return s4p

                    s4s = [s_pair(0, 0), s_pair(0, 1)]
                    for jt in range(N_JT):
                        e4p = [None, None]
                        for p2 in range(2):
                            e4 = ep.tile([112, 2, 448], BF, tag="e", name="e")
                            if _exp_eng(hg, jt, p2) == "A":
                                nc.scalar.activation(
                                    e4[:JT, :, :], s4s[p2][:JT, :, :IC_F], AF.Exp, scale=SCALE
                                )
                            else:
                                nc.vector.tensor_scalar(
                                    e4[:JT, :, :].bitcast(I16),
                                    s4s[p2][:JT, :, :IC_F],
                                    A_EXP,
                                    B_EXP,
                                    ALU.mult,
                                    ALU.add,
                                )
                            e4p[p2] = e4
                        if jt + 1 < N_JT:
                            s4s = [s_pair(jt + 1, 0), None]
                        for p2 in range(2):
                            for hh in (2 * p2, 2 * p2 + 1):
                                tile_, base = o_ps4[hh]
                                nc.tensor.matmul(
                                    tile_[base:base + 64, :],
                                    lhsT=(v_tm[:JT, jt, hg * 4 + hh, :]),
                                    rhs=(e4p[p2][:JT, hh % 2, :]),
                                    start=(jt == 0),
                                    stop=(jt == N_JT - 1),
                                    tile_position=(0, base),
                                    skip_group_check=True,
                                )
                        if jt + 1 < N_JT:
                            s4s[1] = s_pair(jt + 1, 1)

                    # normalize: recip whole psum tiles (d sits replicated in
                    # rows base+32:base+64), then 4 mixed-space multiplies
                    r_f = [rp.tile([128, IC_F], FP, tag=f"rf{t}", name=f"rf{t}") for t in range(2)]
                    nc.vector.reciprocal_approx_fast(r_f[0][:], o_psA[:])
                    nc.vector.reciprocal_approx_fast(r_f[1][:], o_psB[:])
                    for hh in range(4):
                        tile_, base = o_ps4[hh]
                        r_f_t = r_f[0] if hh < 2 else r_f[1]
                        nc.vector.tensor_mul(
                            o_cm[hg][hh * 32:(hh + 1) * 32, ic * IC_F:(ic + 1) * IC_F],
                            tile_[base:base + 32, :],
                            r_f_t[base + 32:base + 64, :],
                        )

                    # after hg0: pw for chunk ic+1; dw chunk ic+2 split so the
                    # DVE queue never has a long burst ahead of exp tiles
                    if hg == 0 and ic + 1 < N_IC:
                        pw_cm("q", q_cm, dwq, (ic + 1) * IC_F, (ic + 2) * IC_F,
                              ((ic + 1) * IC_CH, (ic + 2) * IC_CH), HH)
                        if ic + 2 < N_IC:
                            dw_q(dwq, (ic + 2) * IC_CH, (ic + 3) * IC_CH, cbis=(0,))
                # proj for all fully-covered 128-token blocks
                while ti_ready < n_tt and min(ti_ready * 128, N - 128) + 128 <= (ic + 1) * IC_F:
                    proj_block(min(ti_ready * 128, N - 128))
                    ti_ready += 1
                if ic + 2 < N_IC:
                    dw_q(dwq, (ic + 2) * IC_CH, (ic + 3) * IC_CH, cbis=(1,))

    nc.compile()
    return nc


def _fold_common(inp):
    common = {}
    for p in ("q", "k", "v"):
        scale = inp[f"{p}_bn_g"] / np.sqrt(inp[f"{p}_bn_v"] + EPS)
        shift = inp[f"{p}_bn_b"] - inp[f"{p}_bn_m"] * scale
        const = (
            inp[f"{p}_pw_w"] @ (scale * inp[f"{p}_dw_b"] + shift) + inp[f"{p}_pw_b"]
        ).astype(np.float32)
        if p == "v":
            common["v_const_r1"] = const.reshape(1, C).astype(ml_dtypes.bfloat16)
        else:
            common[f"{p}_const"] = const.reshape(C, 1)
        dw9 = (inp[f"{p}_dw_w"].reshape(C, 9) * scale[:, None]).astype(np.float32)
        common[f"{p}_dww"] = np.ascontiguousarray(
            dw9.reshape(2, 128, 9).transpose(1, 0, 2)
        )
        common[f"{p}_pww"] = np.ascontiguousarray(
            inp[f"{p}_pw_w"].T.reshape(2, 128, C).transpose(1, 0, 2)
        ).astype(ml_dtypes.bfloat16)
    common["proj_wT"] = np.ascontiguousarray(
        inp["proj_w"].T.reshape(2, 128, C)
    ).astype(ml_dtypes.bfloat16)
    common["proj_brep"] = np.ascontiguousarray(
        np.broadcast_to(inp["proj_b"].reshape(1, C), (128, C))
    ).astype(np.float32)
    return common


def prepare_x(xb):
    # xb: (3136, 256) f32 -> padded channel-major bf16 (128, 2, 58, 58)
    xt = xb.T.reshape(C, HH, HH)
    xp = np.zeros((C, PADW, PADW), np.float32)
    xp[:, 1:57, 1:57] = xt
    return np.ascontiguousarray(
        xp.reshape(2, 128, PADW, PADW).transpose(1, 0, 2, 3)
    ).astype(ml_dtypes.bfloat16)


def prepare_in_maps(inp):
    common = _fold_common(inp)
    x = inp["x"].astype(np.float32)
    return [dict(common, x_pad=prepare_x(x[b])) for b in range(x.shape[0])]


def kernel(**inputs):
    inp = {k: np.asarray(v) for k, v in inputs.items()}

    if "nc" not in _CACHED:
        _CACHED["nc"] = _build_nc()
    nc = _CACHED["nc"]

    in_maps = prepare_in_maps(inp)
    res = run_bass_kernel_spmd(nc, in_maps, list(range(len(in_maps))))
    out = np.stack([res.results[b]["out"] for b in range(len(in_maps))], axis=0)
    return out.astype(np.float32)
